# revision 1
# baseline (speedup 1.0000x reference)
"""Multi-head self-attention (B=4, S=2048, E=1024, H=16) on 8 TRN2 NeuronCores.

Sharding: 8 cores = 4 batches x 2 sequence halves. Core c handles batch b=c//2,
query rows [h*1024, (h+1)*1024) with h=c%2. Each core computes Q/K/V for its own
1024-row shard, the K/V shards are exchanged via one 8-rank AllGather (bf16,
K^T and V concatenated in one bounce buffer), and each core then runs full
attention for its 16 heads x 1024 queries over the 2048 keys of its batch,
followed by the full output projection for its rows. Host only shards inputs,
transposes/casts, and concatenates the per-core outputs.

Math notes (exactness-preserving rewrites):
- K bias dropped: adds a per-query constant to every score -> softmax invariant.
- V bias folded into the output-projection bias: bo_eff = WO @ bV + bO.
- 1/sqrt(D) and the additive key mask (-1e6 on masked keys) are fused into the
  exp activation: p = Exp(score/8 + negmask[key]).
- No max-subtraction in softmax: scores are O(1) here, exp cannot overflow.
- Softmax normalizer l rides as a ones-column in the V-hat stationary tiles;
  normalization is applied to the attention output (commutes with per-query
  scaling), via a K=1 ones-matmul that broadcasts 1/l across partitions.
"""

import sys
import os

if "/opt/trn_rl_repo" not in sys.path:
    sys.path.insert(0, "/opt/trn_rl_repo")

import numpy as np
import ml_dtypes

import concourse.bass as bass
import concourse.mybir as mybir
from concourse import bacc
from concourse.tile import TileContext
from concourse.bass_utils import run_bass_kernel_spmd

BF16 = mybir.dt.bfloat16
F32 = mybir.dt.float32

B, S, E, H = 4, 2048, 1024, 16
D = E // H          # 64
N_CORES = 8
ROWS = S // 2       # 1024 query rows per core
KEYS = S            # 2048 keys per core (full batch)
KT = E // 128       # 8 contraction tiles
JT = E // 128       # 8 output-feature tiles
ET = E // 128       # 8 e-tiles (head pairs)
NJC = KEYS // 128   # 16 key chunks
NIC = ROWS // 512   # 2 query chunks of 512
NVT = KEYS // 128   # 16 v key-tiles
NIT = ROWS // 128   # 8 query row-tiles
SCALE = 1.0 / 8.0   # 1/sqrt(D)

_prog_cache = {}


def _build_program(sim=False, loop_n=0):
    """sim=True builds a single-core variant for TimelineSim: the AllGathers are
    dropped and ag_k/ag_v become plain internal DRAM tensors (timing-only).
    loop_n>0 (requires sim=True) wraps the WO phase in a hardware For_i loop
    for wall-clock amplification benchmarks."""
    assert not loop_n or sim
    nc = bacc.Bacc("TRN2", target_bir_lowering=False, debug=False, num_devices=N_CORES)

    xT = nc.dram_tensor("xT", [E, ROWS], BF16, kind="ExternalInput").ap()
    wq = nc.dram_tensor("wq", [E, E], BF16, kind="ExternalInput").ap()
    wk = nc.dram_tensor("wk", [E, E], BF16, kind="ExternalInput").ap()
    wv = nc.dram_tensor("wv", [E, E], BF16, kind="ExternalInput").ap()
    wo = nc.dram_tensor("wo", [E, E], BF16, kind="ExternalInput").ap()
    bq = nc.dram_tensor("bq", [128, JT], F32, kind="ExternalInput").ap()
    negmask = nc.dram_tensor("negmask", [128, NJC], F32, kind="ExternalInput").ap()
    outmask = nc.dram_tensor("outmask", [128, NIT], F32, kind="ExternalInput").ap()
    bo_eff = nc.dram_tensor("bo_eff", [1, E], BF16, kind="ExternalInput").ap()
    pair_base = nc.dram_tensor("pair_base", [1, 1], mybir.dt.uint32, kind="ExternalInput").ap()
    out = nc.dram_tensor("out", [ROWS, E], F32, kind="ExternalOutput").ap()

    with TileContext(nc) as tc:
        with (
            tc.tile_pool(name="persist", bufs=1) as persist,
            tc.tile_pool(name="dram", bufs=1, space="DRAM") as dram,
        ):
            # ---- persistent small tensors ----
            bq_t = persist.tile([128, JT], F32)
            nc.sync.dma_start(out=bq_t, in_=bq[:, :])
            nm_t = persist.tile([128, NJC], F32)
            nc.sync.dma_start(out=nm_t, in_=negmask[:, :])
            om_t = persist.tile([128, NIT], F32)
            nc.sync.dma_start(out=om_t, in_=outmask[:, :])
            ones_t = persist.tile([1, 128], BF16)
            nc.vector.memset(ones_t, 1.0)
            # ---- persistent big tensors ----
            ao_sb = [persist.tile([128, ROWS], BF16, name=f"ao{t}") for t in range(ET)]
            qT_sb = [persist.tile([128, ROWS], BF16, name=f"qT{j}") for j in range(JT)]
            # per-(pair,head) softmax denominators: head hh at partition hh*64
            # (bases 0 and 64 are both legal; footprint = one free-dim range)
            la_sb = [persist.tile([D + 1, ROWS], BF16, name=f"la{t}") for t in range(ET)]

            # ---- bounce buffers for the K/V exchange ----
            addr_space = "Local" if sim else "Shared"
            bounce_k = dram.tile([ROWS, ROWS], BF16)   # own K^T shard (feature-major)
            bounce_v = dram.tile([ROWS, ROWS], BF16)   # own V shard (row-major)
            ag_k = dram.tile([N_CORES * ROWS, ROWS], BF16, addr_space=addr_space)
            ag_v = dram.tile([N_CORES * ROWS, ROWS], BF16, addr_space=addr_space)

            # Key order on this core: [own 1024 | partner 1024] (contraction over
            # keys is permutation-invariant; host reorders negmask to match).
            # Attention runs in two sweeps: sweep A = all pairs x own keys
            # (starts as soon as K(0)/Q(0) exist; K/V/Q production interleaves
            # into the early pairs), sweep B = all pairs x partner keys (the
            # AllGathers complete far behind sweep A's ~130us of work).
            # Sweep A leaves unnormalized partials in ao_sb and raw l in rl_sb;
            # sweep B accumulates, then 1/l is broadcast via a K=2 matmul.
            with (
                tc.tile_pool(name="p_xq", bufs=1) as p_xq,    # x + WQ
                tc.tile_pool(name="p_kst", bufs=8) as p_kst,  # K^T slots (own/partner share)
                tc.tile_pool(name="p_vh", bufs=1) as p_vh,    # all 16 V-hat tiles
                tc.tile_pool(name="p_w", bufs=1) as p_w,      # WK + WV
                tc.tile_pool(name="p2s", bufs=3) as p2s,      # transients
                tc.tile_pool(name="psA", bufs=1, space="PSUM") as psA,
            ):
                xt = [p_xq.tile([128, ROWS], BF16, name=f"xt{k}") for k in range(KT)]
                wo_sb = [p_xq.tile([128, E], BF16, name=f"wo{k}") for k in range(KT)]
                wq_sb = [p_xq.tile([128, E], BF16, name=f"wq{k}") for k in range(KT)]
                kstage = [p_kst.tile([128, ROWS], BF16, name=f"kst{j}", tag="kst") for j in range(JT)]
                vhat = [p_vh.tile([128, H, D + 1], BF16, name=f"vh{v}") for v in range(NVT)]
                wk_sb = [p_w.tile([128, E], BF16, name=f"wk{k}") for k in range(KT)]
                wv_sb = [p_w.tile([128, E], BF16, name=f"wv{k}") for k in range(KT)]
                kpart = [p_kst.tile([128, ROWS], BF16, name=f"kp{j}", tag="kst") for j in range(JT)]

                # load order follows first use: x+WK, WQ, WV
                for k in range(KT):
                    nc.sync.dma_start(out=xt[k], in_=xT[k * 128:(k + 1) * 128, :])
                    nc.sync.dma_start(out=wk_sb[k], in_=wk[k * 128:(k + 1) * 128, :])
                for k in range(KT):
                    nc.sync.dma_start(out=wq_sb[k], in_=wq[k * 128:(k + 1) * 128, :])
                for k in range(KT):
                    nc.sync.dma_start(out=wv_sb[k], in_=wv[k * 128:(k + 1) * 128, :])

                # "s" slots ([128,1024] = 2 PSUM banks x 2) host every transient
                # accumulation: K/V/Q projections, score tiles, normalization.
                # "av" slots (1 bank x 4) host the 4 attn@v chains of a pair.
                def s_tile(shape=None):
                    return psA.tile(shape or [128, ROWS], F32, name="ps_s", tag="s", bufs=2)

                def emit_k(j):
                    for ic in range(NIC):
                        sl = slice(ic * 512, (ic + 1) * 512)
                        ps_k = s_tile([128, 512])
                        for k in range(KT):
                            nc.tensor.matmul(
                                ps_k, wk_sb[k][:, j * 128:(j + 1) * 128], xt[k][:, sl],
                                start=(k == 0), stop=(k == KT - 1),
                            )
                        nc.vector.tensor_copy(kstage[j][:, sl], ps_k)
                    nc.sync.dma_start(out=bounce_k[j * 128:(j + 1) * 128, :], in_=kstage[j])

                def emit_q(j):
                    for ic in range(NIC):
                        sl = slice(ic * 512, (ic + 1) * 512)
                        ps_q = s_tile([128, 512])
                        for k in range(KT):
                            nc.tensor.matmul(
                                ps_q, wq_sb[k][:, j * 128:(j + 1) * 128], xt[k][:, sl],
                                start=(k == 0), stop=(k == KT - 1),
                            )
                        nc.vector.tensor_scalar_add(
                            qT_sb[j][:, sl], ps_q, bq_t[:, j:j + 1]
                        )

                def emit_v(v):
                    # V row-tile v (own keys v*128..): psum -> vhat directly
                    for jc in range(NIC):
                        sl = slice(jc * 512, (jc + 1) * 512)
                        ps_v = s_tile([128, 512])
                        for k in range(KT):
                            nc.tensor.matmul(
                                ps_v, xt[k][:, v * 128:(v + 1) * 128], wv_sb[k][:, sl],
                                start=(k == 0), stop=(k == KT - 1),
                            )
                        nc.vector.tensor_copy(
                            vhat[v][:, 8 * jc:8 * (jc + 1), 0:D],
                            ps_v.rearrange("p (h d) -> p h d", h=8),
                        )
                    nc.vector.memset(vhat[v][:, :, D:D + 1], 1.0)
                    nc.sync.dma_start(
                        out=bounce_v[v * 128:(v + 1) * 128, :],
                        in_=vhat[v][:, :, 0:D],
                    )

                def emit_partner_v(v):
                    vtmp = p2s.tile([128, E], BF16, name="vtmp", tag="vtmp", bufs=2)
                    nc.sync.dma_start(
                        out=vtmp, in_=ag_v[bass.ds(base + v * 128, 128), :]
                    )
                    nc.vector.tensor_copy(
                        vhat[NVT // 2 + v][:, :, 0:D],
                        vtmp.rearrange("p (h d) -> p h d", h=H),
                    )
                    nc.vector.memset(vhat[NVT // 2 + v][:, :, D:D + 1], 1.0)

                def emit_norm(j):
                    # normalization of pair j: 1/l broadcast across partitions
                    # via a K=1 ones-matmul, then scale ao_sb in place.
                    for hh in range(2):
                        for ic in range(NIC):
                            sl = slice(ic * 512, (ic + 1) * 512)
                            r32 = p2s.tile([1, 512], F32, name="r32", tag="r32", bufs=2)
                            nc.vector.reciprocal(r32, la_sb[j][hh * D:hh * D + 1, sl])
                            rbf = p2s.tile([1, 512], BF16, name="rbf", tag="rbf", bufs=2)
                            nc.vector.tensor_copy(rbf, r32)
                            ps_b = s_tile([D, 512])
                            nc.tensor.matmul(
                                ps_b, ones_t[:, 0:D], rbf, start=True, stop=True
                            )
                            nc.vector.tensor_mul(
                                ao_sb[j][hh * D:(hh + 1) * D, sl],
                                ao_sb[j][hh * D:(hh + 1) * D, sl],
                                ps_b,
                            )

                def emit_scores_exp(t, jc, kt_src, kcol):
                    phs = []
                    for hh in range(2):
                        prows = slice(hh * D, (hh + 1) * D)
                        ps_s = s_tile()
                        for ic in range(NIC):
                            nc.tensor.matmul(
                                ps_s[:, ic * 512:(ic + 1) * 512],
                                kt_src[prows, kcol:kcol + 128],
                                qT_sb[t][prows, ic * 512:(ic + 1) * 512],
                                start=True, stop=True,
                                tile_position=(hh * D, 0),
                            )
                        ph = p2s.tile([128, ROWS], BF16, name="ph", tag="ph", bufs=6)
                        nc.scalar.activation(
                            ph, ps_s, mybir.ActivationFunctionType.Exp,
                            bias=nm_t[:, jc:jc + 1], scale=SCALE,
                        )
                        phs.append(ph)
                    return phs

                def emit_av(t, jc, phs, ps_av, first, last):
                    for hh in range(2):
                        h = 2 * t + hh
                        for ic in range(NIC):
                            nc.tensor.matmul(
                                ps_av[hh, ic],
                                vhat[jc][:, h, :],
                                phs[hh][:, ic * 512:(ic + 1) * 512],
                                start=first, stop=last,
                            )

                emit_k(0)
                emit_q(0)
                base_reg = nc.sync.alloc_register("base_reg")
                nc.sync.reg_load(base_reg, pair_base[0:1, 0:1])
                base = nc.sync.snap(
                    base_reg, donate=True, min_val=0, max_val=(N_CORES - 1) * ROWS
                )

                # K(j>=1), Q(j>=1), V(all) interleave into sweep A's early pairs:
                # (pair, jc) -> list of emissions. V(jc) rides pair 0 exactly one
                # LAG ahead of its av consumer; K two per pair in pairs 0-2 (so
                # AG_K fires early); Q(t+2) two pairs ahead of use.
                pre = {}
                pre[(0, 1)] = [("k", 1)]
                pre[(0, 2)] = [("q", 1)]
                for j in range(2, JT):
                    pre.setdefault(((j - 2) // 2, 3 + 4 * ((j - 2) % 2)), []).append(("k", j))
                for j in range(2, JT):
                    pre.setdefault((j - 2, 6), []).append(("q", j))
                for v in range(ROWS // 128):
                    pre.setdefault((0, min(v + 1, NJC // 2 - 1)), []).append(("v", v))

                LAG = 2
                # =============== sweep A: own keys ===============
                for t in range(ET):
                    pend = []
                    ps_av = {}
                    for hh in range(2):
                        for ic in range(NIC):
                            ps_av[hh, ic] = psA.tile(
                                [D + 1, 512], F32, name="ps_av", tag="av", bufs=4
                            )
                    for jc in range(NJC // 2):
                        for kind, idx in pre.get((t, jc), ()):
                            if kind == "k":
                                emit_k(idx)
                                if idx == JT - 1:
                                    if not sim:
                                        nc.gpsimd.collective_compute(
                                            "AllGather", mybir.AluOpType.bypass,
                                            ins=[bounce_k[:, :]], outs=[ag_k[:, :]],
                                            replica_groups=[list(range(N_CORES))],
                                        )
                                    for j in range(JT):
                                        nc.sync.dma_start(
                                            out=kpart[j],
                                            in_=ag_k[bass.ds(base + j * 128, 128), :],
                                        )
                            elif kind == "q":
                                emit_q(idx)
                            else:
                                emit_v(idx)
                                if idx == ROWS // 128 - 1:
                                    if not sim:
                                        nc.gpsimd.collective_compute(
                                            "AllGather", mybir.AluOpType.bypass,
                                            ins=[bounce_v[:, :]], outs=[ag_v[:, :]],
                                            replica_groups=[list(range(N_CORES))],
                                        )
                                    for v in range(NVT // 2):
                                        emit_partner_v(v)
                        pend.append((jc, emit_scores_exp(t, jc, kstage[t], jc * 128)))
                        if len(pend) > LAG:
                            j0, phs0 = pend.pop(0)
                            emit_av(t, j0, phs0, ps_av, j0 == 0, j0 == NJC // 2 - 1)
                    for j0, phs0 in pend:
                        emit_av(t, j0, phs0, ps_av, j0 == 0, j0 == NJC // 2 - 1)
                    # stash unnormalized partials + raw l
                    for hh in range(2):
                        for ic in range(NIC):
                            sl = slice(ic * 512, (ic + 1) * 512)
                            av = ps_av[hh, ic]
                            nc.vector.tensor_copy(
                                la_sb[t][hh * D:hh * D + 1, sl], av[D:D + 1, :]
                            )
                            nc.vector.tensor_copy(
                                ao_sb[t][hh * D:(hh + 1) * D, sl], av[0:D, :]
                            )

                # =============== sweep B: partner keys ===============
                for t in range(ET):
                    pend = []
                    ps_av = {}
                    for hh in range(2):
                        for ic in range(NIC):
                            ps_av[hh, ic] = psA.tile(
                                [D + 1, 512], F32, name="ps_av", tag="av", bufs=4
                            )
                    for jc in range(NJC // 2, NJC):
                        if jc == NJC // 2 + 3 and t > 0:
                            emit_norm(t - 1)  # previous pair: off the boundary
                        pend.append((jc, emit_scores_exp(t, jc, kpart[t], (jc - NJC // 2) * 128)))
                        if len(pend) > LAG:
                            j0, phs0 = pend.pop(0)
                            emit_av(t, j0, phs0, ps_av, j0 == NJC // 2, j0 == NJC - 1)
                    for j0, phs0 in pend:
                        emit_av(t, j0, phs0, ps_av, j0 == NJC // 2, j0 == NJC - 1)
                    # accumulate into partials; l += lB (in place, frees av fast)
                    for hh in range(2):
                        for ic in range(NIC):
                            sl = slice(ic * 512, (ic + 1) * 512)
                            av = ps_av[hh, ic]
                            nc.vector.tensor_add(
                                la_sb[t][hh * D:hh * D + 1, sl],
                                la_sb[t][hh * D:hh * D + 1, sl],
                                av[D:D + 1, :],
                            )
                            nc.vector.tensor_add(
                                ao_sb[t][hh * D:(hh + 1) * D, sl],
                                ao_sb[t][hh * D:(hh + 1) * D, sl],
                                av[0:D, :],
                            )
                emit_norm(ET - 1)

                # WO weights (loaded during attention), then the output
                # projection in the same pool/tag space -- no pool barrier.
                for k in range(KT):
                    nc.sync.dma_start(out=wo_sb[k], in_=wo[k * 128:(k + 1) * 128, :])

                # ======= output projection in the same pools (no barrier) =======
                bo_t = p2s.tile([1, E], BF16, name="bo_t", tag="bo", bufs=1)
                nc.sync.dma_start(out=bo_t, in_=bo_eff[:, :])

                def emit_wo():
                    emit_wo_body(nc, tc, psA, p2s, ao_sb, wo_sb, bo_t, ones_t, om_t, out)

                if loop_n:
                    with tc.For_i(0, loop_n, 1):
                        emit_wo()
                else:
                    emit_wo()
    nc.compile()
    return nc


def emit_wo_body(nc, tc, ps3, p3, ao_sb, wo_sb, bo_t, ones_t, om_t, out):
                for it in range(NIT):
                    for fc in range(NIC):
                        sl = slice(fc * 512, (fc + 1) * 512)
                        ps_o = ps3.tile([128, 512], F32, name="ps_o", tag="av", bufs=4)
                        for k in range(KT):
                            nc.tensor.matmul(
                                ps_o,
                                ao_sb[k][:, it * 128:(it + 1) * 128],
                                wo_sb[k][:, sl],
                                start=(k == 0), stop=False,
                            )
                        nc.tensor.matmul(
                            ps_o, ones_t[:, 0:128], bo_t[:, sl],
                            start=False, stop=True,
                        )
                        o_sb = p3.tile([128, 512], F32, name="o_sb", tag="o_sb", bufs=3)
                        nc.scalar.activation(
                            o_sb, ps_o, mybir.ActivationFunctionType.Abs,
                            scale=om_t[:, it:it + 1],
                        )
                        nc.sync.dma_start(
                            out=out[it * 128:(it + 1) * 128, sl], in_=o_sb
                        )


def _make_executor():
    """Build the Bass program once and wrap it in a cached jitted shard_map
    (adapted from concourse.bass2jax.run_bass_via_pjrt, hoisting the jit out
    of the per-call path so repeat calls don't retrace/recompile)."""
    import jax
    from jax.experimental.shard_map import shard_map
    from jax.sharding import Mesh, PartitionSpec, NamedSharding
    from concourse.bass2jax import (
        _bass_exec_p,
        install_neuronx_cc_hook,
        partition_id_tensor,
    )

    nc = _build_program()
    install_neuronx_cc_hook()
    assert nc.dbg_addr is None
    partition_name = nc.partition_id_tensor.name if nc.partition_id_tensor else None

    in_names, out_names, out_avals, zero_outs = [], [], [], []
    for alloc in nc.m.functions[0].allocations:
        if not isinstance(alloc, mybir.MemoryLocationSet):
            continue
        name = alloc.memorylocations[0].name
        if alloc.kind == "ExternalInput":
            if name != partition_name:
                in_names.append(name)
        elif alloc.kind == "ExternalOutput":
            shape = tuple(alloc.tensor_shape)
            dtype = mybir.dt.np(alloc.dtype)
            out_names.append(name)
            out_avals.append(jax.core.ShapedArray(shape, dtype))
            zero_outs.append(np.zeros(shape, dtype))
    n_params = len(in_names)
    n_outs = len(out_avals)
    all_names = in_names + out_names
    if partition_name is not None:
        all_names = all_names + [partition_name]
    donate = tuple(range(n_params, n_params + n_outs))

    def _body(*args):
        operands = list(args)
        if partition_name is not None:
            operands.append(partition_id_tensor())
        outs = _bass_exec_p.bind(
            *operands,
            out_avals=tuple(out_avals),
            in_names=tuple(all_names),
            out_names=tuple(out_names),
            lowering_input_output_aliases=(),
            sim_require_finite=True,
            sim_require_nnan=True,
            nc=nc,
        )
        return tuple(outs)

    devices = jax.devices()[:N_CORES]
    mesh = Mesh(np.asarray(devices), ("core",))
    in_specs = (PartitionSpec("core"),) * (n_params + n_outs)
    out_specs = (PartitionSpec("core"),) * n_outs
    sharded = jax.jit(
        shard_map(_body, mesh=mesh, in_specs=in_specs, out_specs=out_specs,
                  check_rep=False),
        donate_argnums=donate,
        keep_unused=True,
    )
    sharding = NamedSharding(mesh, PartitionSpec("core"))
    return {
        "jit": sharded, "in_names": in_names, "out_names": out_names,
        "out_avals": out_avals, "zero_outs": zero_outs, "sharding": sharding,
        "jax": jax,
    }


def get_executor():
    if "ex" not in _prog_cache:
        _prog_cache["ex"] = _make_executor()
    return _prog_cache["ex"]


def run_spmd(in_maps):
    """Execute on 8 cores; returns list of per-core output dicts."""
    ex = get_executor()
    jax = ex["jax"]
    concat_in = [
        np.concatenate([np.asarray(m[name]) for m in in_maps], axis=0)
        for name in ex["in_names"]
    ]
    concat_zeros = [
        np.zeros((N_CORES * z.shape[0], *z.shape[1:]), z.dtype)
        for z in ex["zero_outs"]
    ]
    out_arrs = ex["jit"](*concat_in, *concat_zeros)
    return [
        {
            name: np.asarray(out_arrs[i]).reshape(N_CORES, *ex["out_avals"][i].shape)[c]
            for i, name in enumerate(ex["out_names"])
        }
        for c in range(N_CORES)
    ]


def build_in_maps(x, mask, WQ_w, WQ_b, WK_w, WK_b, WV_w, WV_b, WO_w, WO_b):
    x = np.asarray(x, dtype=np.float32)
    mask = np.asarray(mask).astype(bool)
    WQ_w = np.asarray(WQ_w, dtype=np.float32)
    WQ_b = np.asarray(WQ_b, dtype=np.float32)
    WK_w = np.asarray(WK_w, dtype=np.float32)
    WV_w = np.asarray(WV_w, dtype=np.float32)
    WV_b = np.asarray(WV_b, dtype=np.float32)
    WO_w = np.asarray(WO_w, dtype=np.float32)
    WO_b = np.asarray(WO_b, dtype=np.float32)

    wq_t = np.ascontiguousarray(WQ_w.T).astype(ml_dtypes.bfloat16)
    wk_t = np.ascontiguousarray(WK_w.T).astype(ml_dtypes.bfloat16)
    wv_t = np.ascontiguousarray(WV_w.T).astype(ml_dtypes.bfloat16)
    wo_t = np.ascontiguousarray(WO_w.T).astype(ml_dtypes.bfloat16)
    bq_t = np.ascontiguousarray(WQ_b.reshape(JT, 128).T)  # [128, JT] f32
    bo_eff = (WO_w @ WV_b + WO_b).astype(ml_dtypes.bfloat16).reshape(1, E)

    in_maps = []
    for c in range(N_CORES):
        b, h = divmod(c, 2)
        x_sh = x[b, h * ROWS:(h + 1) * ROWS, :]                      # (1024, 1024)
        xT_sh = np.ascontiguousarray(x_sh.T).astype(ml_dtypes.bfloat16)
        # key order on this core: [own half | partner half]
        mask_perm = np.concatenate(
            [mask[b, h * ROWS:(h + 1) * ROWS], mask[b, (1 - h) * ROWS:(2 - h) * ROWS]]
        )
        negmask = np.where(mask_perm, 0.0, -1e6).astype(np.float32)
        nm_t = np.ascontiguousarray(negmask.reshape(NJC, 128).T)     # [128, 16]
        om = mask[b, h * ROWS:(h + 1) * ROWS].astype(np.float32)
        om_t = np.ascontiguousarray(om.reshape(NIT, 128).T)          # [128, 8]
        in_maps.append({
            "xT": xT_sh, "wq": wq_t, "wk": wk_t, "wv": wv_t, "wo": wo_t,
            "bq": bq_t, "negmask": nm_t, "outmask": om_t, "bo_eff": bo_eff,
            "pair_base": np.array([[(c ^ 1) * ROWS]], dtype=np.uint32),
        })
    return in_maps


def kernel(x, mask, WQ_w, WQ_b, WK_w, WK_b, WV_w, WV_b, WO_w, WO_b):
    mask = np.asarray(mask).astype(bool)
    in_maps = build_in_maps(x, mask, WQ_w, WQ_b, WK_w, WK_b, WV_w, WV_b, WO_w, WO_b)
    results = run_spmd(in_maps)
    out = np.empty((B, S, E), dtype=np.float32)
    for c in range(N_CORES):
        b, h = divmod(c, 2)
        out[b, h * ROWS:(h + 1) * ROWS, :] = results[c]["out"]
    return out



# revision 4
# speedup vs baseline: 2.4056x; 2.4056x over previous
"""Multi-head self-attention (B=4, S=2048, E=1024, H=16) on 8 TRN2 NeuronCores.

Sharding: 8 cores = 4 batches x 2 sequence halves. Core c handles batch b=c//2,
query rows [h*1024, (h+1)*1024) with h=c%2. Each core receives the FULL batch x
(2048 rows, feature-major, reordered [own half | partner half]) and computes
K/V for all 2048 keys locally -- no collectives. This doubles the K/V
projection FLOPs (~55us) but removes the two AllGathers, which cost ~8.5ms of
wall time in this axon-tunneled environment. Q and the output projection cover
only the core's own 1024 query rows. Host only shards/reorders inputs,
transposes/casts, and concatenates the per-core outputs.

Math notes (exactness-preserving rewrites):
- K bias dropped: adds a per-query constant to every score -> softmax invariant.
- V bias folded into the output-projection bias: bo_eff = WO @ bV + bO.
- 1/sqrt(D) and the additive key mask (-1e6 on masked keys) are fused into the
  exp activation: p = Exp(score/8 + negmask[key]).
- No max-subtraction in softmax: scores are O(1) here, exp cannot overflow.
- Softmax normalizer l rides as a ones-column in the V-hat stationary tiles;
  normalization is applied to the attention output (commutes with per-query
  scaling), via a K=1 ones-matmul that broadcasts 1/l across partitions.
"""

import sys
import os

if "/opt/trn_rl_repo" not in sys.path:
    sys.path.insert(0, "/opt/trn_rl_repo")

import numpy as np
import ml_dtypes

import concourse.bass as bass
import concourse.mybir as mybir
from concourse import bacc
from concourse.tile import TileContext

BF16 = mybir.dt.bfloat16
F32 = mybir.dt.float32

B, S, E, H = 4, 2048, 1024, 16
D = E // H          # 64
N_CORES = 8
ROWS = S // 2       # 1024 query rows per core
KEYS = S            # 2048 keys per core (full batch)
KT = E // 128       # 8 contraction tiles
JT = E // 128       # 8 output-feature tiles
ET = E // 128       # 8 e-tiles (head pairs)
NJC = KEYS // 128   # 16 key chunks
NIC = ROWS // 512   # 2 query chunks of 512
NKC = KEYS // 512   # 4 key chunks of 512 (K projection)
NVT = KEYS // 128   # 16 v key-tiles
NIT = ROWS // 128   # 8 query row-tiles
SCALE = 1.0 / 8.0   # 1/sqrt(D)

_prog_cache = {}


def _build_program(sim=False, loop_n=0):
    """sim has no effect (kept for tooling compat); loop_n>0 wraps the WO
    phase in a hardware For_i loop for wall-clock amplification benchmarks."""
    nc = bacc.Bacc("TRN2", target_bir_lowering=False, debug=False, num_devices=N_CORES)

    xT = nc.dram_tensor("xT", [E, KEYS], BF16, kind="ExternalInput").ap()
    wq = nc.dram_tensor("wq", [E, E], BF16, kind="ExternalInput").ap()
    wk = nc.dram_tensor("wk", [E, E], BF16, kind="ExternalInput").ap()
    wv = nc.dram_tensor("wv", [E, E], BF16, kind="ExternalInput").ap()
    wo = nc.dram_tensor("wo", [E, E], BF16, kind="ExternalInput").ap()
    bq = nc.dram_tensor("bq", [128, JT], F32, kind="ExternalInput").ap()
    negmask = nc.dram_tensor("negmask", [128, NJC], F32, kind="ExternalInput").ap()
    outmask = nc.dram_tensor("outmask", [128, NIT], F32, kind="ExternalInput").ap()
    bo_eff = nc.dram_tensor("bo_eff", [1, E], BF16, kind="ExternalInput").ap()
    out = nc.dram_tensor("out", [ROWS, E], F32, kind="ExternalOutput").ap()

    with TileContext(nc) as tc:
        with tc.tile_pool(name="persist", bufs=1) as persist:
            # ---- persistent small tensors ----
            bq_t = persist.tile([128, JT], F32)
            nc.sync.dma_start(out=bq_t, in_=bq[:, :])
            nm_t = persist.tile([128, NJC], F32)
            nc.sync.dma_start(out=nm_t, in_=negmask[:, :])
            om_t = persist.tile([128, NIT], F32)
            nc.sync.dma_start(out=om_t, in_=outmask[:, :])
            ones_t = persist.tile([1, 128], BF16)
            nc.vector.memset(ones_t, 1.0)
            # ---- persistent big tensors ----
            ao_sb = [persist.tile([128, ROWS], BF16, name=f"ao{t}") for t in range(ET)]

            with (
                tc.tile_pool(name="p_x", bufs=1) as p_x,      # x (full batch)
                tc.tile_pool(name="p_kst", bufs=3) as p_kst,  # K^T rotating slots
                tc.tile_pool(name="p_q", bufs=3) as p_q,      # Q^T rotating slots
                tc.tile_pool(name="p_vh", bufs=1) as p_vh,    # all 16 V-hat tiles
                tc.tile_pool(name="p_w", bufs=1) as p_w,      # weights
                tc.tile_pool(name="p2s", bufs=3) as p2s,      # transients
                tc.tile_pool(name="psA", bufs=1, space="PSUM") as psA,
            ):
                xt = [p_x.tile([128, KEYS], BF16, name=f"xt{k}") for k in range(KT)]
                kstage = [p_kst.tile([128, KEYS], BF16, name=f"kst{j}", tag="kst")
                          for j in range(JT)]
                qT_sb = [p_q.tile([128, ROWS], BF16, name=f"qT{j}", tag="qt")
                         for j in range(JT)]
                vhat = [p_vh.tile([128, H, D + 1], BF16, name=f"vh{v}") for v in range(NVT)]
                wk_sb = [p_w.tile([128, E], BF16, name=f"wk{k}") for k in range(KT)]
                wv_sb = [p_w.tile([128, E], BF16, name=f"wv{k}", tag=f"wvo{k}") for k in range(KT)]
                wq_sb = [p_w.tile([128, E], BF16, name=f"wq{k}") for k in range(KT)]
                # wo_sb[k] reuses wv_sb[k]'s slot once V production is done
                wo_sb = [p_w.tile([128, E], BF16, name=f"wo{k}", tag=f"wvo{k}") for k in range(KT)]

                # load order follows first use: x+WK, WQ, WV
                for k in range(KT):
                    nc.sync.dma_start(out=xt[k], in_=xT[k * 128:(k + 1) * 128, :])
                    nc.sync.dma_start(out=wk_sb[k], in_=wk[k * 128:(k + 1) * 128, :])
                for k in range(KT):
                    nc.sync.dma_start(out=wq_sb[k], in_=wq[k * 128:(k + 1) * 128, :])
                for k in range(KT):
                    nc.sync.dma_start(out=wv_sb[k], in_=wv[k * 128:(k + 1) * 128, :])

                # "s" slots ([128,1024] = 2 PSUM banks x 2) host every transient
                # accumulation: K/V/Q projections, score tiles, normalization.
                # "av" slots (1 bank x 4) host the 4 attn@v chains of a pair.
                def s_tile(shape=None):
                    return psA.tile(shape or [128, ROWS], F32, name="ps_s", tag="s", bufs=2)

                def emit_k(j):
                    # K^T feature rows [j*128, (j+1)*128) over all 2048 keys
                    for kc in range(NKC):
                        sl = slice(kc * 512, (kc + 1) * 512)
                        ps_k = s_tile([128, 512])
                        for k in range(KT):
                            nc.tensor.matmul(
                                ps_k, wk_sb[k][:, j * 128:(j + 1) * 128], xt[k][:, sl],
                                start=(k == 0), stop=(k == KT - 1),
                            )
                        nc.vector.tensor_copy(kstage[j][:, sl], ps_k)

                def emit_q(j):
                    for ic in range(NIC):
                        sl = slice(ic * 512, (ic + 1) * 512)
                        ps_q = s_tile([128, 512])
                        for k in range(KT):
                            nc.tensor.matmul(
                                ps_q, wq_sb[k][:, j * 128:(j + 1) * 128],
                                xt[k][:, ic * 512:(ic + 1) * 512],
                                start=(k == 0), stop=(k == KT - 1),
                            )
                        nc.vector.tensor_scalar_add(
                            qT_sb[j][:, sl], ps_q, bq_t[:, j:j + 1]
                        )

                def emit_v(v):
                    # V row-tile v (keys v*128..(v+1)*128): psum -> vhat
                    for jc in range(NIC):
                        sl = slice(jc * 512, (jc + 1) * 512)
                        ps_v = s_tile([128, 512])
                        for k in range(KT):
                            nc.tensor.matmul(
                                ps_v, xt[k][:, v * 128:(v + 1) * 128], wv_sb[k][:, sl],
                                start=(k == 0), stop=(k == KT - 1),
                            )
                        nc.vector.tensor_copy(
                            vhat[v][:, 8 * jc:8 * (jc + 1), 0:D],
                            ps_v.rearrange("p (h d) -> p h d", h=8),
                        )
                    nc.vector.memset(vhat[v][:, :, D:D + 1], 1.0)

                def emit_norm(t, ps_av):
                    # normalization of pair t: stash av -> SBUF, then scale by
                    # 1/l broadcast across partitions via a K=1 ones-matmul
                    # (DVE cannot read two PSUM operands in one op).
                    for hh in range(2):
                        for ic in range(NIC):
                            sl = slice(ic * 512, (ic + 1) * 512)
                            av = ps_av[hh, ic]
                            r32 = p2s.tile([1, 512], F32, name="r32", tag="r32", bufs=2)
                            nc.vector.reciprocal(r32, av[D:D + 1, :])
                            rbf = p2s.tile([1, 512], BF16, name="rbf", tag="rbf", bufs=2)
                            nc.vector.tensor_copy(rbf, r32)
                            nc.vector.tensor_copy(
                                ao_sb[t][hh * D:(hh + 1) * D, sl], av[0:D, :]
                            )
                            ps_b = s_tile([D, 512])
                            nc.tensor.matmul(
                                ps_b, ones_t[:, 0:D], rbf, start=True, stop=True
                            )
                            nc.vector.tensor_mul(
                                ao_sb[t][hh * D:(hh + 1) * D, sl],
                                ao_sb[t][hh * D:(hh + 1) * D, sl],
                                ps_b,
                            )

                def emit_scores_exp(t, jc):
                    phs = []
                    for hh in range(2):
                        prows = slice(hh * D, (hh + 1) * D)
                        ps_s = s_tile()
                        for ic in range(NIC):
                            nc.tensor.matmul(
                                ps_s[:, ic * 512:(ic + 1) * 512],
                                kstage[t][prows, jc * 128:(jc + 1) * 128],
                                qT_sb[t][prows, ic * 512:(ic + 1) * 512],
                                start=True, stop=True,
                                tile_position=(hh * D, 0),
                            )
                        ph = p2s.tile([128, ROWS], BF16, name="ph", tag="ph", bufs=8)
                        nc.scalar.activation(
                            ph, ps_s, mybir.ActivationFunctionType.Exp,
                            bias=nm_t[:, jc:jc + 1], scale=SCALE,
                        )
                        phs.append(ph)
                    return phs

                def emit_av(t, jc, phs, ps_av, first, last):
                    for hh in range(2):
                        h = 2 * t + hh
                        for ic in range(NIC):
                            nc.tensor.matmul(
                                ps_av[hh, ic],
                                vhat[jc][:, h, :],
                                phs[hh][:, ic * 512:(ic + 1) * 512],
                                start=first, stop=last,
                            )

                emit_k(0)
                emit_q(0)

                # Production of K(j>=1), Q(j>=1), V(all) interleaves into the
                # sweep: (pair, jc) -> list of emissions. V(v) rides pair 0 one
                # chunk ahead of its av consumer (av lags scores by LAG); K/Q
                # for pair t+1 are produced mid-pair t.
                pre = {}
                pre[(0, 1)] = [("k", 1)]
                pre[(0, 3)] = [("q", 1)]
                for j in range(2, JT):
                    pre.setdefault((j - 1, 8), []).append(("k", j))
                for j in range(2, JT):
                    pre.setdefault((j - 1, 12), []).append(("q", j))
                for v in range(NVT):
                    pre.setdefault((0, min(v + 1, NJC - 1)), []).append(("v", v))

                LAG = 2
                for t in range(ET):
                    pend = []
                    ps_av = {}
                    for hh in range(2):
                        for ic in range(NIC):
                            ps_av[hh, ic] = psA.tile(
                                [D + 1, 512], F32, name="ps_av", tag="av", bufs=4
                            )
                    lag = 4 if t == 0 else LAG
                    for jc in range(NJC):
                        for kind, idx in pre.get((t, jc), ()):
                            if kind == "k":
                                emit_k(idx)
                            elif kind == "q":
                                emit_q(idx)
                            else:
                                emit_v(idx)
                        pend.append((jc, emit_scores_exp(t, jc)))
                        if len(pend) > lag:
                            j0, phs0 = pend.pop(0)
                            emit_av(t, j0, phs0, ps_av, j0 == 0, j0 == NJC - 1)
                    for j0, phs0 in pend:
                        emit_av(t, j0, phs0, ps_av, j0 == 0, j0 == NJC - 1)
                    emit_norm(t, ps_av)

                # WO weights (loaded during attention, into wv's slots), then
                # the output projection in the same pool/tag space.
                for k in range(KT):
                    nc.sync.dma_start(out=wo_sb[k], in_=wo[k * 128:(k + 1) * 128, :])

                bo_t = p2s.tile([1, E], BF16, name="bo_t", tag="bo", bufs=1)
                nc.sync.dma_start(out=bo_t, in_=bo_eff[:, :])

                def emit_wo():
                    emit_wo_body(nc, tc, psA, p2s, ao_sb, wo_sb, bo_t, ones_t, om_t, out)

                if loop_n:
                    with tc.For_i(0, loop_n, 1):
                        emit_wo()
                else:
                    emit_wo()
    nc.compile()
    return nc


def emit_wo_body(nc, tc, ps3, p3, ao_sb, wo_sb, bo_t, ones_t, om_t, out):
    for it in range(NIT):
        for fc in range(NIC):
            sl = slice(fc * 512, (fc + 1) * 512)
            ps_o = ps3.tile([128, 512], F32, name="ps_o", tag="av", bufs=4)
            for k in range(KT):
                nc.tensor.matmul(
                    ps_o,
                    ao_sb[k][:, it * 128:(it + 1) * 128],
                    wo_sb[k][:, sl],
                    start=(k == 0), stop=False,
                )
            nc.tensor.matmul(
                ps_o, ones_t[:, 0:128], bo_t[:, sl],
                start=False, stop=True,
            )
            o_sb = p3.tile([128, 512], F32, name="o_sb", tag="o_sb", bufs=3)
            nc.scalar.activation(
                o_sb, ps_o, mybir.ActivationFunctionType.Abs,
                scale=om_t[:, it:it + 1],
            )
            nc.sync.dma_start(
                out=out[it * 128:(it + 1) * 128, sl], in_=o_sb
            )


def _make_executor():
    """Build the Bass program once and wrap it in a cached jitted shard_map
    (adapted from concourse.bass2jax.run_bass_via_pjrt, hoisting the jit out
    of the per-call path so repeat calls don't retrace/recompile)."""
    import jax
    from jax.experimental.shard_map import shard_map
    from jax.sharding import Mesh, PartitionSpec, NamedSharding
    from concourse.bass2jax import (
        _bass_exec_p,
        install_neuronx_cc_hook,
        partition_id_tensor,
    )

    nc = _build_program()
    install_neuronx_cc_hook()
    assert nc.dbg_addr is None
    partition_name = nc.partition_id_tensor.name if nc.partition_id_tensor else None

    in_names, out_names, out_avals, zero_outs = [], [], [], []
    for alloc in nc.m.functions[0].allocations:
        if not isinstance(alloc, mybir.MemoryLocationSet):
            continue
        name = alloc.memorylocations[0].name
        if alloc.kind == "ExternalInput":
            if name != partition_name:
                in_names.append(name)
        elif alloc.kind == "ExternalOutput":
            shape = tuple(alloc.tensor_shape)
            dtype = mybir.dt.np(alloc.dtype)
            out_names.append(name)
            out_avals.append(jax.core.ShapedArray(shape, dtype))
            zero_outs.append(np.zeros(shape, dtype))
    n_params = len(in_names)
    n_outs = len(out_avals)
    all_names = in_names + out_names
    if partition_name is not None:
        all_names = all_names + [partition_name]
    donate = tuple(range(n_params, n_params + n_outs))

    def _body(*args):
        operands = list(args)
        if partition_name is not None:
            operands.append(partition_id_tensor())
        outs = _bass_exec_p.bind(
            *operands,
            out_avals=tuple(out_avals),
            in_names=tuple(all_names),
            out_names=tuple(out_names),
            lowering_input_output_aliases=(),
            sim_require_finite=True,
            sim_require_nnan=True,
            nc=nc,
        )
        return tuple(outs)

    devices = jax.devices()[:N_CORES]
    mesh = Mesh(np.asarray(devices), ("core",))
    in_specs = (PartitionSpec("core"),) * (n_params + n_outs)
    out_specs = (PartitionSpec("core"),) * n_outs
    sharded = jax.jit(
        shard_map(_body, mesh=mesh, in_specs=in_specs, out_specs=out_specs,
                  check_rep=False),
        donate_argnums=donate,
        keep_unused=True,
    )
    sharding = NamedSharding(mesh, PartitionSpec("core"))
    return {
        "jit": sharded, "in_names": in_names, "out_names": out_names,
        "out_avals": out_avals, "zero_outs": zero_outs, "sharding": sharding,
        "jax": jax,
    }


def get_executor():
    if "ex" not in _prog_cache:
        _prog_cache["ex"] = _make_executor()
    return _prog_cache["ex"]


def run_spmd(in_maps):
    """Execute on 8 cores; returns list of per-core output dicts."""
    ex = get_executor()
    jax = ex["jax"]
    concat_in = [
        np.concatenate([np.asarray(m[name]) for m in in_maps], axis=0)
        for name in ex["in_names"]
    ]
    concat_zeros = [
        np.zeros((N_CORES * z.shape[0], *z.shape[1:]), z.dtype)
        for z in ex["zero_outs"]
    ]
    out_arrs = ex["jit"](*concat_in, *concat_zeros)
    return [
        {
            name: np.asarray(out_arrs[i]).reshape(N_CORES, *ex["out_avals"][i].shape)[c]
            for i, name in enumerate(ex["out_names"])
        }
        for c in range(N_CORES)
    ]


def build_in_maps(x, mask, WQ_w, WQ_b, WK_w, WK_b, WV_w, WV_b, WO_w, WO_b):
    x = np.asarray(x, dtype=np.float32)
    mask = np.asarray(mask).astype(bool)
    WQ_w = np.asarray(WQ_w, dtype=np.float32)
    WQ_b = np.asarray(WQ_b, dtype=np.float32)
    WK_w = np.asarray(WK_w, dtype=np.float32)
    WV_w = np.asarray(WV_w, dtype=np.float32)
    WV_b = np.asarray(WV_b, dtype=np.float32)
    WO_w = np.asarray(WO_w, dtype=np.float32)
    WO_b = np.asarray(WO_b, dtype=np.float32)

    wq_t = np.ascontiguousarray(WQ_w.T).astype(ml_dtypes.bfloat16)
    wk_t = np.ascontiguousarray(WK_w.T).astype(ml_dtypes.bfloat16)
    wv_t = np.ascontiguousarray(WV_w.T).astype(ml_dtypes.bfloat16)
    wo_t = np.ascontiguousarray(WO_w.T).astype(ml_dtypes.bfloat16)
    bq_t = np.ascontiguousarray(WQ_b.reshape(JT, 128).T)  # [128, JT] f32
    bo_eff = (WO_w @ WV_b + WO_b).astype(ml_dtypes.bfloat16).reshape(1, E)

    in_maps = []
    for c in range(N_CORES):
        b, h = divmod(c, 2)
        # full batch x, rows reordered [own half | partner half], feature-major
        x_perm = np.concatenate(
            [x[b, h * ROWS:(h + 1) * ROWS, :], x[b, (1 - h) * ROWS:(2 - h) * ROWS, :]]
        )
        xT_sh = np.ascontiguousarray(x_perm.T).astype(ml_dtypes.bfloat16)
        mask_perm = np.concatenate(
            [mask[b, h * ROWS:(h + 1) * ROWS], mask[b, (1 - h) * ROWS:(2 - h) * ROWS]]
        )
        negmask = np.where(mask_perm, 0.0, -1e6).astype(np.float32)
        nm_t = np.ascontiguousarray(negmask.reshape(NJC, 128).T)     # [128, 16]
        om = mask[b, h * ROWS:(h + 1) * ROWS].astype(np.float32)
        om_t = np.ascontiguousarray(om.reshape(NIT, 128).T)          # [128, 8]
        in_maps.append({
            "xT": xT_sh, "wq": wq_t, "wk": wk_t, "wv": wv_t, "wo": wo_t,
            "bq": bq_t, "negmask": nm_t, "outmask": om_t, "bo_eff": bo_eff,
        })
    return in_maps


def kernel(x, mask, WQ_w, WQ_b, WK_w, WK_b, WV_w, WV_b, WO_w, WO_b):
    mask = np.asarray(mask).astype(bool)
    in_maps = build_in_maps(x, mask, WQ_w, WQ_b, WK_w, WK_b, WV_w, WV_b, WO_w, WO_b)
    results = run_spmd(in_maps)
    out = np.empty((B, S, E), dtype=np.float32)
    for c in range(N_CORES):
        b, h = divmod(c, 2)
        out[b, h * ROWS:(h + 1) * ROWS, :] = results[c]["out"]
    return out


# revision 6
# speedup vs baseline: 2.4275x; 1.0091x over previous
"""Multi-head self-attention (B=4, S=2048, E=1024, H=16) on 8 TRN2 NeuronCores.

Sharding: 8 cores = 4 batches x 2 sequence halves. Core c handles batch b=c//2,
query rows [h*1024, (h+1)*1024) with h=c%2. No collectives: each core receives
the x rows it needs for the FULL batch and computes K/V locally (the two
AllGathers this replaces cost ~8.5ms of wall time in this axon environment).

Mask compaction: the key mask zeroes ~half the keys (exp(score-1e6) == 0.0 in
f32, exactly) and the output mask zeroes ~half the query rows. The host
gathers only valid keys/queries into a compacted, padded layout:

    x columns per core: [own-half valid rows | pad->NQ | partner valid | pad->NK]

so queries are columns [0, NQ) and keys are all NK columns. Pad columns carry
x=0 and negmask=-1e6 (their softmax weight is exactly 0); pad query rows are
discarded by the host scatter. NQ/NK are static per compiled program; the
executor is cached per (NQ, NK) shape, so repeat calls with the same mask
density reuse the compiled NEFF.

Math notes (exactness-preserving rewrites):
- K bias dropped: adds a per-query constant to every score -> softmax invariant.
- V bias folded into the output-projection bias: bo_eff = WO @ bV + bO.
- 1/sqrt(D) and the additive key mask are fused into the exp activation:
  p = Exp(score/8 + negmask[key]).
- No max-subtraction in softmax: scores are O(1) here, exp cannot overflow.
- Softmax normalizer l rides as a ones-column in the V-hat stationary tiles;
  normalization is applied to the attention output (commutes with per-query
  scaling), via a K=1 ones-matmul that broadcasts 1/l across partitions.
- The final abs(o * mask) needs no mask multiply: every kept query row has
  mask=1, so it reduces to Abs.
"""

import sys

if "/opt/trn_rl_repo" not in sys.path:
    sys.path.insert(0, "/opt/trn_rl_repo")

import numpy as np
import ml_dtypes

import concourse.bass as bass
import concourse.mybir as mybir
from concourse import bacc
from concourse.tile import TileContext

BF16 = mybir.dt.bfloat16
F32 = mybir.dt.float32

B, S, E, H = 4, 2048, 1024, 16
D = E // H          # 64
N_CORES = 8
ROWS = S // 2       # 1024 query rows owned per core
KT = E // 128       # 8 contraction tiles
JT = E // 128       # 8 output-feature tiles
ET = E // 128       # 8 e-tiles (head pairs)
SCALE = 1.0 / 8.0   # 1/sqrt(D)

_prog_cache = {}


def _round_up(n, m):
    return ((n + m - 1) // m) * m


def _pads_from_mask(mask):
    """(NQ, NK): padded query/key counts shared by all 8 cores."""
    mask = np.asarray(mask).astype(bool)
    counts = [mask[b, h * ROWS:(h + 1) * ROWS].sum() for b in range(B) for h in range(2)]
    nq = _round_up(max(int(c) for c in counts), 128)
    nq = max(nq, 256)
    # partner count for core (b,h) is counts of (b,1-h); max is the same set
    npart = nq
    return nq, nq + npart


def _build_program(NQ, NK, loop_n=0):
    """Build the compacted-attention program for padded query count NQ and
    padded key count NK (keys = [queries | partner]). loop_n>0 wraps the WO
    phase in a hardware For_i loop for wall-clock amplification benchmarks."""
    NJC = NK // 128     # key chunks (scores / av / vhat)
    NVT = NK // 128
    NIT = NQ // 128     # output row tiles
    assert NQ % 128 == 0 and NK % 128 == 0 and NQ <= 1024
    # chunks must not cross PSUM 512-f32 bank boundaries
    kchunks = [(s, min(512, NK - s)) for s in range(0, NK, 512)]
    qchunks = [(s, min(512, NQ - s)) for s in range(0, NQ, 512)]
    NQC = len(qchunks)

    nc = bacc.Bacc("TRN2", target_bir_lowering=False, debug=False, num_devices=N_CORES)

    xT = nc.dram_tensor("xT", [E, NK], BF16, kind="ExternalInput").ap()
    wq = nc.dram_tensor("wq", [E, E], BF16, kind="ExternalInput").ap()
    wk = nc.dram_tensor("wk", [E, E], BF16, kind="ExternalInput").ap()
    wv = nc.dram_tensor("wv", [E, E], BF16, kind="ExternalInput").ap()
    wo = nc.dram_tensor("wo", [E, E], BF16, kind="ExternalInput").ap()
    bq = nc.dram_tensor("bq", [128, JT], F32, kind="ExternalInput").ap()
    negmask = nc.dram_tensor("negmask", [128, NJC], F32, kind="ExternalInput").ap()
    bo_eff = nc.dram_tensor("bo_eff", [1, E], BF16, kind="ExternalInput").ap()
    out = nc.dram_tensor("out", [NQ, E], F32, kind="ExternalOutput").ap()

    with TileContext(nc) as tc:
        with tc.tile_pool(name="persist", bufs=1) as persist:
            bq_t = persist.tile([128, JT], F32)
            nc.sync.dma_start(out=bq_t, in_=bq[:, :])
            nm_t = persist.tile([128, NJC], F32)
            nc.sync.dma_start(out=nm_t, in_=negmask[:, :])
            ones_t = persist.tile([1, 128], BF16)
            nc.vector.memset(ones_t, 1.0)
            ao_sb = [persist.tile([128, NQ], BF16, name=f"ao{t}") for t in range(ET)]

            with (
                tc.tile_pool(name="p_x", bufs=1) as p_x,
                tc.tile_pool(name="p_kst", bufs=3) as p_kst,
                tc.tile_pool(name="p_q", bufs=3) as p_q,
                tc.tile_pool(name="p_vh", bufs=1) as p_vh,
                tc.tile_pool(name="p_w", bufs=1) as p_w,
                tc.tile_pool(name="p2s", bufs=3) as p2s,
                tc.tile_pool(name="psA", bufs=1, space="PSUM") as psA,
            ):
                xt = [p_x.tile([128, NK], BF16, name=f"xt{k}") for k in range(KT)]
                kstage = [p_kst.tile([128, NK], BF16, name=f"kst{j}", tag="kst")
                          for j in range(JT)]
                qT_sb = [p_q.tile([128, NQ], BF16, name=f"qT{j}", tag="qt")
                         for j in range(JT)]
                vhat = [p_vh.tile([128, H, D + 1], BF16, name=f"vh{v}") for v in range(NVT)]
                wk_sb = [p_w.tile([128, E], BF16, name=f"wk{k}") for k in range(KT)]
                wv_sb = [p_w.tile([128, E], BF16, name=f"wv{k}", tag=f"wvo{k}") for k in range(KT)]
                wq_sb = [p_w.tile([128, E], BF16, name=f"wq{k}") for k in range(KT)]
                # wo_sb[k] reuses wv_sb[k]'s slot once V production is done
                wo_sb = [p_w.tile([128, E], BF16, name=f"wo{k}", tag=f"wvo{k}") for k in range(KT)]

                for k in range(KT):
                    nc.sync.dma_start(out=xt[k], in_=xT[k * 128:(k + 1) * 128, :])
                    nc.sync.dma_start(out=wk_sb[k], in_=wk[k * 128:(k + 1) * 128, :])
                for k in range(KT):
                    nc.sync.dma_start(out=wq_sb[k], in_=wq[k * 128:(k + 1) * 128, :])
                for k in range(KT):
                    nc.sync.dma_start(out=wv_sb[k], in_=wv[k * 128:(k + 1) * 128, :])

                def s_tile(shape=None):
                    return psA.tile(shape or [128, NQ], F32, name="ps_s", tag="s", bufs=2)

                def emit_k(j):
                    for (st, sz) in kchunks:
                        sl = slice(st, st + sz)
                        ps_k = s_tile([128, sz])
                        for k in range(KT):
                            nc.tensor.matmul(
                                ps_k, wk_sb[k][:, j * 128:(j + 1) * 128], xt[k][:, sl],
                                start=(k == 0), stop=(k == KT - 1),
                            )
                        nc.vector.tensor_copy(kstage[j][:, sl], ps_k)

                def emit_q(j):
                    for (st, sz) in qchunks:
                        sl = slice(st, st + sz)
                        ps_q = s_tile([128, sz])
                        for k in range(KT):
                            nc.tensor.matmul(
                                ps_q, wq_sb[k][:, j * 128:(j + 1) * 128], xt[k][:, sl],
                                start=(k == 0), stop=(k == KT - 1),
                            )
                        nc.vector.tensor_scalar_add(
                            qT_sb[j][:, sl], ps_q, bq_t[:, j:j + 1]
                        )

                def emit_v(v):
                    for jc in range(2):
                        sl = slice(jc * 512, (jc + 1) * 512)
                        ps_v = s_tile([128, 512])
                        for k in range(KT):
                            nc.tensor.matmul(
                                ps_v, xt[k][:, v * 128:(v + 1) * 128], wv_sb[k][:, sl],
                                start=(k == 0), stop=(k == KT - 1),
                            )
                        nc.vector.tensor_copy(
                            vhat[v][:, 8 * jc:8 * (jc + 1), 0:D],
                            ps_v.rearrange("p (h d) -> p h d", h=8),
                        )
                    nc.vector.memset(vhat[v][:, :, D:D + 1], 1.0)

                def emit_norm(t, ps_av):
                    # stash av -> SBUF, then scale by 1/l broadcast across
                    # partitions via a K=1 ones-matmul (DVE cannot read two
                    # PSUM operands in one op).
                    for hh in range(2):
                        for ic, (st, sz) in enumerate(qchunks):
                            sl = slice(st, st + sz)
                            av = ps_av[hh, ic]
                            r32 = p2s.tile([1, sz], F32, name="r32", tag="r32", bufs=2)
                            nc.vector.reciprocal(r32, av[D:D + 1, :])
                            rbf = p2s.tile([1, sz], BF16, name="rbf", tag="rbf", bufs=2)
                            nc.vector.tensor_copy(rbf, r32)
                            nc.vector.tensor_copy(
                                ao_sb[t][hh * D:(hh + 1) * D, sl], av[0:D, :]
                            )
                            ps_b = s_tile([D, sz])
                            nc.tensor.matmul(
                                ps_b, ones_t[:, 0:D], rbf, start=True, stop=True
                            )
                            nc.vector.tensor_mul(
                                ao_sb[t][hh * D:(hh + 1) * D, sl],
                                ao_sb[t][hh * D:(hh + 1) * D, sl],
                                ps_b,
                            )

                def emit_scores_exp(t, jc):
                    phs = []
                    for hh in range(2):
                        prows = slice(hh * D, (hh + 1) * D)
                        ps_s = s_tile()
                        for (st, sz) in qchunks:
                            nc.tensor.matmul(
                                ps_s[:, st:st + sz],
                                kstage[t][prows, jc * 128:(jc + 1) * 128],
                                qT_sb[t][prows, st:st + sz],
                                start=True, stop=True,
                                tile_position=(hh * D, 0),
                            )
                        ph = p2s.tile([128, NQ], BF16, name="ph", tag="ph", bufs=10)
                        nc.scalar.activation(
                            ph, ps_s, mybir.ActivationFunctionType.Exp,
                            bias=nm_t[:, jc:jc + 1], scale=SCALE,
                        )
                        phs.append(ph)
                    return phs

                def emit_av(t, jc, phs, ps_av, first, last):
                    for hh in range(2):
                        h = 2 * t + hh
                        for ic, (st, sz) in enumerate(qchunks):
                            nc.tensor.matmul(
                                ps_av[hh, ic],
                                vhat[jc][:, h, :],
                                phs[hh][:, st:st + sz],
                                start=first, stop=last,
                            )

                emit_k(0)
                emit_q(0)

                # Production of K(j>=1), Q(j>=1), V(all) interleaves into the
                # sweep: (pair, jc) -> list of emissions.
                pre = {}
                pre[(0, 1)] = [("k", 1)]
                pre[(0, 3)] = [("q", 1)]
                for j in range(2, JT):
                    pre.setdefault((j - 1, NJC // 2), []).append(("k", j))
                for j in range(2, JT):
                    pre.setdefault((j - 1, 3 * NJC // 4), []).append(("q", j))
                for v in range(NVT):
                    pre.setdefault((0, min(v + 1, NJC - 1)), []).append(("v", v))

                LAG = 2
                for t in range(ET):
                    pend = []
                    ps_av = {}
                    for hh in range(2):
                        for ic, (st, sz) in enumerate(qchunks):
                            ps_av[hh, ic] = psA.tile(
                                [D + 1, sz], F32, name="ps_av", tag="av", bufs=4
                            )
                    lag = 4 if t == 0 else LAG
                    for jc in range(NJC):
                        for kind, idx in pre.get((t, jc), ()):
                            if kind == "k":
                                emit_k(idx)
                            elif kind == "q":
                                emit_q(idx)
                            else:
                                emit_v(idx)
                        pend.append((jc, emit_scores_exp(t, jc)))
                        if len(pend) > lag:
                            j0, phs0 = pend.pop(0)
                            emit_av(t, j0, phs0, ps_av, j0 == 0, j0 == NJC - 1)
                    for j0, phs0 in pend:
                        emit_av(t, j0, phs0, ps_av, j0 == 0, j0 == NJC - 1)
                    emit_norm(t, ps_av)

                for k in range(KT):
                    nc.sync.dma_start(out=wo_sb[k], in_=wo[k * 128:(k + 1) * 128, :])

                bo_t = p2s.tile([1, E], BF16, name="bo_t", tag="bo", bufs=1)
                nc.sync.dma_start(out=bo_t, in_=bo_eff[:, :])

                def emit_wo():
                    for it in range(NIT):
                        for fc in range(2):
                            sl = slice(fc * 512, (fc + 1) * 512)
                            ps_o = psA.tile([128, 512], F32, name="ps_o", tag="av", bufs=4)
                            for k in range(KT):
                                nc.tensor.matmul(
                                    ps_o,
                                    ao_sb[k][:, it * 128:(it + 1) * 128],
                                    wo_sb[k][:, sl],
                                    start=(k == 0), stop=False,
                                )
                            nc.tensor.matmul(
                                ps_o, ones_t[:, 0:128], bo_t[:, sl],
                                start=False, stop=True,
                            )
                            o_sb = p2s.tile([128, 512], F32, name="o_sb", tag="o_sb", bufs=3)
                            nc.scalar.activation(
                                o_sb, ps_o, mybir.ActivationFunctionType.Abs,
                            )
                            nc.sync.dma_start(
                                out=out[it * 128:(it + 1) * 128, sl], in_=o_sb
                            )

                if loop_n:
                    with tc.For_i(0, loop_n, 1):
                        emit_wo()
                else:
                    emit_wo()
    nc.compile()
    return nc


def _make_executor(NQ, NK):
    """Build the Bass program once and wrap it in a cached jitted shard_map
    (adapted from concourse.bass2jax.run_bass_via_pjrt, hoisting the jit out
    of the per-call path so repeat calls don't retrace/recompile)."""
    import jax
    from jax.experimental.shard_map import shard_map
    from jax.sharding import Mesh, PartitionSpec, NamedSharding
    from concourse.bass2jax import (
        _bass_exec_p,
        install_neuronx_cc_hook,
        partition_id_tensor,
    )

    nc = _build_program(NQ, NK)
    install_neuronx_cc_hook()
    assert nc.dbg_addr is None
    partition_name = nc.partition_id_tensor.name if nc.partition_id_tensor else None

    in_names, out_names, out_avals, zero_outs = [], [], [], []
    for alloc in nc.m.functions[0].allocations:
        if not isinstance(alloc, mybir.MemoryLocationSet):
            continue
        name = alloc.memorylocations[0].name
        if alloc.kind == "ExternalInput":
            if name != partition_name:
                in_names.append(name)
        elif alloc.kind == "ExternalOutput":
            shape = tuple(alloc.tensor_shape)
            dtype = mybir.dt.np(alloc.dtype)
            out_names.append(name)
            out_avals.append(jax.core.ShapedArray(shape, dtype))
            zero_outs.append(np.zeros(shape, dtype))
    n_params = len(in_names)
    n_outs = len(out_avals)
    all_names = in_names + out_names
    if partition_name is not None:
        all_names = all_names + [partition_name]
    donate = tuple(range(n_params, n_params + n_outs))

    def _body(*args):
        operands = list(args)
        if partition_name is not None:
            operands.append(partition_id_tensor())
        outs = _bass_exec_p.bind(
            *operands,
            out_avals=tuple(out_avals),
            in_names=tuple(all_names),
            out_names=tuple(out_names),
            lowering_input_output_aliases=(),
            sim_require_finite=True,
            sim_require_nnan=True,
            nc=nc,
        )
        return tuple(outs)

    devices = jax.devices()[:N_CORES]
    mesh = Mesh(np.asarray(devices), ("core",))
    in_specs = (PartitionSpec("core"),) * (n_params + n_outs)
    out_specs = (PartitionSpec("core"),) * n_outs
    sharded = jax.jit(
        shard_map(_body, mesh=mesh, in_specs=in_specs, out_specs=out_specs,
                  check_rep=False),
        donate_argnums=donate,
        keep_unused=True,
    )
    sharding = NamedSharding(mesh, PartitionSpec("core"))
    return {
        "jit": sharded, "in_names": in_names, "out_names": out_names,
        "out_avals": out_avals, "zero_outs": zero_outs, "sharding": sharding,
        "jax": jax, "NQ": NQ, "NK": NK,
    }


def get_executor(NQ=None, NK=None):
    if NQ is None:
        NQ, NK = _prog_cache["last_pads"]
    key = ("ex", NQ, NK)
    if key not in _prog_cache:
        _prog_cache[key] = _make_executor(NQ, NK)
    _prog_cache["last_pads"] = (NQ, NK)
    return _prog_cache[key]


def run_spmd(in_maps, NQ, NK):
    """Execute on 8 cores; returns list of per-core output dicts."""
    ex = get_executor(NQ, NK)
    jax = ex["jax"]
    concat_in = [
        np.concatenate([np.asarray(m[name]) for m in in_maps], axis=0)
        for name in ex["in_names"]
    ]
    concat_zeros = [
        np.zeros((N_CORES * z.shape[0], *z.shape[1:]), z.dtype)
        for z in ex["zero_outs"]
    ]
    out_arrs = ex["jit"](*concat_in, *concat_zeros)
    return [
        {
            name: np.asarray(out_arrs[i]).reshape(N_CORES, *ex["out_avals"][i].shape)[c]
            for i, name in enumerate(ex["out_names"])
        }
        for c in range(N_CORES)
    ]


def build_in_maps(x, mask, WQ_w, WQ_b, WK_w, WK_b, WV_w, WV_b, WO_w, WO_b):
    x = np.asarray(x, dtype=np.float32)
    mask = np.asarray(mask).astype(bool)
    WQ_w = np.asarray(WQ_w, dtype=np.float32)
    WQ_b = np.asarray(WQ_b, dtype=np.float32)
    WK_w = np.asarray(WK_w, dtype=np.float32)
    WV_w = np.asarray(WV_w, dtype=np.float32)
    WV_b = np.asarray(WV_b, dtype=np.float32)
    WO_w = np.asarray(WO_w, dtype=np.float32)
    WO_b = np.asarray(WO_b, dtype=np.float32)

    NQ, NK = _pads_from_mask(mask)
    _prog_cache["last_pads"] = (NQ, NK)
    NJC = NK // 128

    wq_t = np.ascontiguousarray(WQ_w.T).astype(ml_dtypes.bfloat16)
    wk_t = np.ascontiguousarray(WK_w.T).astype(ml_dtypes.bfloat16)
    wv_t = np.ascontiguousarray(WV_w.T).astype(ml_dtypes.bfloat16)
    wo_t = np.ascontiguousarray(WO_w.T).astype(ml_dtypes.bfloat16)
    bq_t = np.ascontiguousarray(WQ_b.reshape(JT, 128).T)  # [128, JT] f32
    bo_eff = (WO_w @ WV_b + WO_b).astype(ml_dtypes.bfloat16).reshape(1, E)

    in_maps = []
    for c in range(N_CORES):
        b, h = divmod(c, 2)
        own_idx = np.nonzero(mask[b, h * ROWS:(h + 1) * ROWS])[0] + h * ROWS
        par_idx = np.nonzero(mask[b, (1 - h) * ROWS:(2 - h) * ROWS])[0] + (1 - h) * ROWS
        # compacted x columns: [own valid | pad->NQ | partner valid | pad->NK]
        xc = np.zeros((NK, E), np.float32)
        xc[0:len(own_idx)] = x[b, own_idx, :]
        xc[NQ:NQ + len(par_idx)] = x[b, par_idx, :]
        xT_sh = np.ascontiguousarray(xc.T).astype(ml_dtypes.bfloat16)
        valid = np.zeros(NK, bool)
        valid[0:len(own_idx)] = True
        valid[NQ:NQ + len(par_idx)] = True
        negmask = np.where(valid, 0.0, -1e6).astype(np.float32)
        nm_t = np.ascontiguousarray(negmask.reshape(NJC, 128).T)     # [128, NJC]
        in_maps.append({
            "xT": xT_sh, "wq": wq_t, "wk": wk_t, "wv": wv_t, "wo": wo_t,
            "bq": bq_t, "negmask": nm_t, "bo_eff": bo_eff,
            "_own_idx": own_idx,   # host-side only (stripped before device)
        })
    return in_maps


def kernel(x, mask, WQ_w, WQ_b, WK_w, WK_b, WV_w, WV_b, WO_w, WO_b):
    mask = np.asarray(mask).astype(bool)
    in_maps = build_in_maps(x, mask, WQ_w, WQ_b, WK_w, WK_b, WV_w, WV_b, WO_w, WO_b)
    NQ, NK = _prog_cache["last_pads"]
    results = run_spmd(in_maps, NQ, NK)
    out = np.zeros((B, S, E), dtype=np.float32)
    for c in range(N_CORES):
        b, h = divmod(c, 2)
        own_idx = in_maps[c]["_own_idx"]
        out[b, own_idx, :] = results[c]["out"][0:len(own_idx)]
    return out
